# revision 8
# baseline (speedup 1.0000x reference)
"""ConvLSTM (BN + conv1 + 16-step ConvLSTM cell) on 8 Trainium2 NeuronCores.

Sharding: data-parallel over (batch n, H-half) -> 8 shards.  The T-recurrence
is sequential, so each core runs its shard's full recurrence with 16 redundant
H rows (48 of 64): garbage creeps one row per step from the shared edge, and
the 32 owned rows stay valid through all 16 steps -> zero inter-core traffic
in the recurrence.  The only collective is a tiny AllGather of BN partial
sums.

BatchNorm is folded into conv1: bnx = s*x + b per channel, so conv1(bnx) =
conv1_{w*s}(x) + conv(b*validmask) + conv1_b; the last two terms form a
precomputed bias map B.  Convs run as shifted-tap fp32r matmul accumulations;
conv1 packs tap rows (dy=0,dy=1) in one K=128 matmul via a row-shifted second
copy of the input on partitions 64..127 (dy=2 rides zero-padded weight rows),
cutting conv1 matmuls from 9 to 6 per gate group.

Partition layout (walrus requires same start partition on all DVE operands
and dst partition 0 on fp32r matmuls): state h on partitions 0:64, c on
64:128; the fo gate group is ordered [o|f] so f lands on 64:128 next to c;
the g gate is duplicated onto both halves (free-dim-bound matmul, so free);
tanh(c_new) crosses from the upper to the lower half via an identity matmul
(K rows 64:128 -> dst partitions 0:64) through PSUM.
"""
import numpy as np

import concourse.bass as bass
import concourse.tile as tile
from concourse import mybir
from concourse.bass_utils import run_bass_kernel_spmd

F32 = mybir.dt.float32
F32R = mybir.dt.float32r
U32 = mybir.dt.uint32
AX = mybir.AxisListType
ALU = mybir.AluOpType
ACTF = mybir.ActivationFunctionType

T, NB, C, H, W = 16, 4, 64, 64, 64
G2 = 256           # per-tap weight block: [o|f] (128) + [g|g] (128)
PR, PW = 50, 66    # padded rows / cols of the on-chip buffers
FT = PR * PW       # 3300
RW = 48            # LSTM rows computed per core
RBS = (1, 9, 17, 25, 33, 41)   # chunk start rows (padded coords), 8 rows each
EPS = 1e-5
CNT = float(T * NB * H * W)    # per-channel BN count
N_CORES = 8
MAX_WAITS = 1      # walrus in this container rejects >1 sync wait per inst


def _split_excess_waits(nc):
    for bb in nc.main_func.blocks:
        new_insts = []
        changed = False
        for inst in bb.instructions:
            si = inst.sync_info
            waits = list(si.on_wait) if (si is not None and si.on_wait) else []
            if len(waits) > MAX_WAITS:
                changed = True
                for w in waits[MAX_WAITS:]:
                    ev = mybir.InstEventSemaphore(
                        name=nc.get_next_instruction_name(),
                        engine=inst.engine,
                        ins=[], outs=[],
                        sync_info=mybir.SyncInfo(on_wait=[w], on_update=[]),
                        bass_nofuse=True,
                    )
                    nc.register_instruction(ev)
                    new_insts.append(ev)
                inst.sync_info = mybir.SyncInfo(
                    on_wait=waits[:MAX_WAITS], on_update=list(si.on_update or [])
                )
            new_insts.append(inst)
        if changed:
            bb.instructions = new_insts


def _emit_conv6(nc, pp, src3, wmat, drain_fo, drain_g, tag, n):
    """conv1-style conv, 6 matmuls per gate group.  src3: [128, PR, PW] view
    (copyA on partitions 0:64, row-shifted copyB on 64:128).  wmat: [128, 6*G2]
    SBUF tile.  drain_fo/drain_g(rb, ps3) emit the PSUM drain for one chunk."""
    for ci, rb in enumerate(RBS):
        psfo = pp.tile([128, 512], F32, tag="lsfo", name=f"pso_{tag}_{n}_{ci}")
        psg = pp.tile([128, 512], F32, tag="lsg", name=f"psg_{tag}_{n}_{ci}")
        for half in range(2):
            ps = psfo if half == 0 else psg
            for k in range(6):
                if k < 3:
                    rhs = src3[:, rb - 1:rb + 7, k:k + 64]
                else:
                    rhs = src3[:, rb:rb + 8, k - 3:k - 3 + 64]
                lhsT = wmat[:, k * G2 + 128 * half:k * G2 + 128 * (half + 1)]
                nc.tensor.matmul(ps[:], lhsT.bitcast(F32R), rhs.bitcast(F32R),
                                 start=(k == 0), stop=(k == 5))
        drain_fo(rb, psfo.rearrange("p (r w) -> p r w", w=64))
        drain_g(rb, psg.rearrange("p (r w) -> p r w", w=64))


def build_nc(n_cores=N_CORES, n_steps=T):
    nc = bass.Bass("TRN2", target_bir_lowering=False, debug=False,
                   num_devices=n_cores)
    xs_d = nc.dram_tensor("xs", [T, C, PR, W], F32, kind="ExternalInput")
    xst_d = nc.dram_tensor("xst", [T, C, 32, W], F32, kind="ExternalInput")
    w1_d = nc.dram_tensor("w1", [128, 6 * G2], F32, kind="ExternalInput")
    wl_d = nc.dram_tensor("wl", [128, 9 * G2], F32, kind="ExternalInput")
    b1_d = nc.dram_tensor("b1", [256, 1], F32, kind="ExternalInput")
    gb_d = nc.dram_tensor("gb", [2, C], F32, kind="ExternalInput")
    msk_d = nc.dram_tensor("msk", [128, FT], F32, kind="ExternalInput")
    idm_d = nc.dram_tensor("idm", [128, 64], F32, kind="ExternalInput")
    hs_d = nc.dram_tensor("hs", [T, C, RW, W], F32, kind="ExternalOutput")

    with tile.TileContext(nc, num_cores=n_cores) as tc:
        with (
            tc.tile_pool(name="const", bufs=1) as cp,
            tc.tile_pool(name="x2p", bufs=2) as xp,
            tc.tile_pool(name="gp", bufs=4) as gp,
            tc.tile_pool(name="ps", bufs=4, space="PSUM") as pp,
            tc.tile_pool(name="dr", bufs=1, space="DRAM") as dp,
        ):
            # ---------------- Phase A: BN partial stats ----------------
            sums8 = cp.tile([128, 8], F32, name="sums8")
            sq8 = cp.tile([128, 8], F32, name="sq8")
            with tc.tile_pool(name="stp", bufs=2) as sp:
                for q in range(8):
                    xq = sp.tile([128, 2048], F32, tag="xq", name=f"xq{q}")
                    nc.gpsimd.dma_start(xq[0:64, :],
                                        xst_d[2 * q].rearrange("c r w -> c (r w)"))
                    nc.gpsimd.dma_start(xq[64:128, :],
                                        xst_d[2 * q + 1].rearrange("c r w -> c (r w)"))
                    nc.vector.reduce_sum(sums8[:, q:q + 1], xq[:], axis=AX.X)
                    trash = sp.tile([128, 2048], F32, tag="trash", bufs=1,
                                    name=f"tr{q}")
                    nc.scalar.activation(trash[:], xq[:], ACTF.Square,
                                         accum_out=sq8[:, q:q + 1])
            pk = cp.tile([128, 2], F32, name="pk")
            nc.vector.reduce_sum(pk[:, 0:1], sums8[:], axis=AX.X)
            nc.vector.reduce_sum(pk[:, 1:2], sq8[:], axis=AX.X)

            cc_in = dp.tile([128, 2], F32, name="cc_in")
            cc_out = dp.tile([128 * n_cores, 2], F32, addr_space="Shared",
                             name="cc_out")
            nc.gpsimd.dma_start(cc_in[:], pk[:])
            nc.gpsimd.collective_compute(
                "AllGather", ALU.bypass,
                ins=[cc_in.opt()], outs=[cc_out.opt()],
                replica_groups=[list(range(n_cores))],
            )
            # st: [c, (j, k)] with j in {sum, sumsq}, k = 2*n_cores copies
            nk = 2 * n_cores
            st = cp.tile([64, 2 * nk], F32, name="st")
            nc.gpsimd.dma_start(
                st.rearrange("p (j k) -> p j k", j=2),
                cc_out.rearrange("(k c) j -> c j k", c=64))
            sums_all = cp.tile([64, 1], F32, name="sums_all")
            sq_all = cp.tile([64, 1], F32, name="sq_all")
            nc.vector.reduce_sum(sums_all[:], st[:, 0:nk], axis=AX.X)
            nc.vector.reduce_sum(sq_all[:], st[:, nk:2 * nk], axis=AX.X)

            mean = cp.tile([64, 1], F32, name="mean")
            nc.vector.tensor_scalar_mul(mean[:], sums_all[:], 1.0 / CNT)
            ex2 = cp.tile([64, 1], F32, name="ex2")
            nc.vector.tensor_scalar_mul(ex2[:], sq_all[:], 1.0 / CNT)
            var = cp.tile([64, 1], F32, name="var")
            nc.vector.tensor_tensor(out=var[:], in0=mean[:], in1=mean[:],
                                    op=ALU.mult)
            nc.vector.tensor_tensor(out=var[:], in0=ex2[:], in1=var[:],
                                    op=ALU.subtract)
            nc.vector.tensor_scalar_add(var[:], var[:], EPS)
            sd = cp.tile([64, 1], F32, name="sd")
            nc.scalar.activation(sd[:], var[:], ACTF.Sqrt)
            inv = cp.tile([64, 1], F32, name="inv")
            nc.vector.reciprocal(inv[:], sd[:])

            gbt = cp.tile([64, 2], F32, name="gbt")
            nc.gpsimd.dma_start(gbt[:], gb_d.rearrange("j c -> c j"))
            s_t = cp.tile([64, 1], F32, name="s_t")
            nc.vector.tensor_tensor(out=s_t[:], in0=inv[:], in1=gbt[:, 0:1],
                                    op=ALU.mult)
            b_t = cp.tile([64, 1], F32, name="b_t")
            nc.vector.tensor_tensor(out=b_t[:], in0=mean[:], in1=s_t[:],
                                    op=ALU.mult)
            nc.vector.tensor_tensor(out=b_t[:], in0=gbt[:, 1:2], in1=b_t[:],
                                    op=ALU.subtract)
            s_rep = cp.tile([128, 1], F32, name="s_rep")
            nc.gpsimd.dma_start(s_rep[0:64, :], s_t[:])
            nc.gpsimd.dma_start(s_rep[64:128, :], s_t[:])
            b_rep = cp.tile([128, 1], F32, name="b_rep")
            nc.gpsimd.dma_start(b_rep[0:64, :], b_t[:])
            nc.gpsimd.dma_start(b_rep[64:128, :], b_t[:])

            # ---------------- Phase B: weights + bias map ----------------
            w1s = cp.tile([128, 6 * G2], F32, name="w1s")
            nc.gpsimd.dma_start(w1s[:].bitcast(F32R), w1_d[:].bitcast(F32R))
            nc.vector.tensor_scalar_mul(w1s[:].bitcast(F32R), w1s[:],
                                        s_rep[:])
            wlt = cp.tile([128, 9 * G2], F32, name="wlt")
            nc.gpsimd.dma_start(wlt[:].bitcast(F32R), wl_d[:].bitcast(F32R))
            idm = cp.tile([128, 64], F32, name="idm")
            nc.gpsimd.dma_start(idm[:].bitcast(F32R), idm_d[:].bitcast(F32R))
            b1fo = cp.tile([128, 1], F32, name="b1fo")
            nc.gpsimd.dma_start(b1fo[:], b1_d[0:128, :])
            b1g = cp.tile([128, 1], F32, name="b1g")
            nc.gpsimd.dma_start(b1g[:], b1_d[128:256, :])

            mskt = xp.tile([128, FT], F32, tag="x2", name="mskt")
            nc.gpsimd.dma_start(mskt[:], msk_d[:])
            pb = xp.tile([128, FT], F32, tag="x2", name="pb")
            nc.gpsimd.memset(pb[:].bitcast(U32), 0)
            nc.vector.tensor_scalar_mul(pb[:].bitcast(F32R), mskt[:],
                                        b_rep[:])

            bfo = cp.tile([128, FT], F32, name="bfo")
            nc.gpsimd.memset(bfo[:], 0.0)
            bg = cp.tile([128, FT], F32, name="bg")
            nc.gpsimd.memset(bg[:], 0.0)
            bfo3 = bfo.rearrange("p (r w) -> p r w", w=PW)
            bg3 = bg.rearrange("p (r w) -> p r w", w=PW)

            def bdrain_fo(rb, ps3):
                nc.vector.tensor_scalar_add(bfo3[:, rb:rb + 8, 1:65], ps3,
                                            b1fo[:])

            def bdrain_g(rb, ps3):
                nc.vector.tensor_scalar_add(bg3[64:128, rb:rb + 8, 1:65],
                                            ps3[64:128], b1g[64:128])

            _emit_conv6(nc, pp, pb.rearrange("p (r w) -> p r w", w=PW), w1s,
                        bdrain_fo, bdrain_g, "b", 0)

            # ---------------- Phase C: state init ----------------
            s_a = cp.tile([128, FT], F32, name="s_a")
            s_b = cp.tile([128, FT], F32, name="s_b")
            nc.gpsimd.memset(s_a[:].bitcast(U32), 0)
            nc.gpsimd.memset(s_b[:].bitcast(U32), 0)
            sts = [s_a.rearrange("p (r w) -> p r w", w=PW),
                   s_b.rearrange("p (r w) -> p r w", w=PW)]

            x2s = [None] * n_steps

            def load_x2(t):
                x2 = xp.tile([128, FT], F32, tag="x2", name=f"x2_{t}")
                x23 = x2.rearrange("p (r w) -> p r w", w=PW)
                nc.vector.memset(x23[:, :, 0:1].bitcast(U32), 0)
                nc.vector.memset(x23[:, :, 65:66].bitcast(U32), 0)
                nc.gpsimd.dma_start(x23[0:64, 0:PR, 1:65].bitcast(F32R),
                                    xs_d[t % T].bitcast(F32R))
                nc.gpsimd.dma_start(x23[64:128, 0:PR - 1, 1:65].bitcast(F32R),
                                    xs_d[t % T, :, 1:PR, :].bitcast(F32R))
                x2s[t] = x23

            def emit_step(t):
                sc3, sn3 = sts[t % 2], sts[(t + 1) % 2]
                x23 = x2s[t]
                for pi in range(3):
                    cis = (2 * pi, 2 * pi + 1)
                    pss, sigs, tgs, cns, tccs = [], [], [], [], []
                    for ci in cis:
                        rb = RBS[ci]
                        psfo = pp.tile([128, 512], F32, tag="lsfo",
                                       name=f"lfo_{t}_{ci}")
                        psg = pp.tile([128, 512], F32, tag="lsg",
                                      name=f"lg_{t}_{ci}")
                        for half in range(2):
                            ps = psfo if half == 0 else psg
                            # conv1 taps on x_t (opens the accumulation group)
                            for k in range(6):
                                if k < 3:
                                    rhs = x23[:, rb - 1:rb + 7, k:k + 64]
                                else:
                                    rhs = x23[:, rb:rb + 8, k - 3:k - 3 + 64]
                                lhsT = w1s[:, k * G2 + 128 * half:
                                           k * G2 + 128 * (half + 1)]
                                nc.tensor.matmul(ps[:], lhsT.bitcast(F32R),
                                                 rhs.bitcast(F32R),
                                                 start=(k == 0), stop=False)
                            # lstm taps on the state
                            for tau in range(9):
                                dy, dx = tau // 3, tau % 3
                                rhs = sc3[:, rb + dy - 1:rb + dy + 7,
                                          dx:dx + 64].bitcast(F32R)
                                lhsT = wlt[:, tau * G2 + 128 * half:
                                           tau * G2 + 128 * (half + 1)]
                                nc.tensor.matmul(ps[:], lhsT.bitcast(F32R),
                                                 rhs, start=False,
                                                 stop=(tau == 8))
                        ps3 = psfo.rearrange("p (r w) -> p r w", w=64)
                        pg3 = psg.rearrange("p (r w) -> p r w", w=64)
                        nc.vector.tensor_tensor(out=ps3[:], in0=ps3[:],
                                                in1=bfo3[:, rb:rb + 8, 1:65],
                                                op=ALU.add)
                        nc.vector.tensor_tensor(out=pg3[64:128],
                                                in0=pg3[64:128],
                                                in1=bg3[64:128, rb:rb + 8, 1:65],
                                                op=ALU.add)
                        pss.append((psfo, psg))
                    # ACT phase: sigmoids together, then tanhs (fewer table
                    # switches), with the DVE c-chain between the tanh groups
                    for j, ci in enumerate(cis):
                        sig = gp.tile([128, 512], F32, tag="sig",
                                      name=f"sig_{t}_{ci}")
                        nc.scalar.activation(sig[:], pss[j][0][:], ACTF.Sigmoid)
                        sigs.append(sig.rearrange("p (r w) -> p r w", w=64))
                    for j, ci in enumerate(cis):
                        tg = gp.tile([128, 512], F32, tag="tg",
                                     name=f"tg_{t}_{ci}")
                        nc.scalar.activation(tg[64:128, :], pss[j][1][64:128, :],
                                             ACTF.Tanh)
                        tgs.append(tg.rearrange("p (r w) -> p r w", w=64))
                    for j, ci in enumerate(cis):
                        rb = RBS[ci]
                        cn = gp.tile([128, 512], F32, tag="cn",
                                     name=f"cn_{t}_{ci}")
                        cn3 = cn.rearrange("p (r w) -> p r w", w=64)
                        nc.vector.tensor_tensor(out=cn3[64:128].bitcast(F32R),
                                                in0=sc3[64:128, rb:rb + 8, 1:65],
                                                in1=tgs[j][64:128],
                                                op=ALU.subtract)
                        nc.vector.tensor_tensor(out=cn3[64:128].bitcast(F32R),
                                                in0=sigs[j][64:128],
                                                in1=cn3[64:128], op=ALU.mult)
                        nc.vector.tensor_tensor(out=cn3[64:128].bitcast(F32R),
                                                in0=cn3[64:128],
                                                in1=tgs[j][64:128], op=ALU.add)
                        nc.vector.tensor_copy(
                            sn3[64:128, rb:rb + 8, 1:65].bitcast(F32R),
                            cn3[64:128])
                        nc.tensor.matmul(pss[j][1][0:64, :],
                                         idm[64:128, :].bitcast(F32R),
                                         cn[64:128, :].bitcast(F32R),
                                         start=True, stop=True)
                        cns.append(cn)
                    for j, ci in enumerate(cis):
                        tcc = gp.tile([128, 512], F32, tag="tcc",
                                      name=f"tcc_{t}_{ci}")
                        nc.scalar.activation(tcc[0:64, :], pss[j][1][0:64, :],
                                             ACTF.Tanh)
                        tccs.append(tcc.rearrange("p (r w) -> p r w", w=64))
                    for j, ci in enumerate(cis):
                        rb = RBS[ci]
                        nc.vector.tensor_tensor(
                            out=sn3[0:64, rb:rb + 8, 1:65].bitcast(F32R),
                            in0=sigs[j][0:64], in1=tccs[j][0:64],
                            op=ALU.mult)
                nc.gpsimd.dma_start(hs_d[t % T], sn3[0:64, 1:49, 1:65])

            load_x2(0)
            for t in range(n_steps):
                if t + 1 < n_steps:
                    load_x2(t + 1)
                emit_step(t)

    _split_excess_waits(nc)
    return nc


def host_prep(x, gamma, beta, conv1_w, conv1_b, w_h2h, w_c2h):
    x = np.asarray(x, np.float32)
    conv1_w = np.asarray(conv1_w, np.float32)
    conv1_b = np.asarray(conv1_b, np.float32)
    w_h2h = np.asarray(w_h2h, np.float32)
    w_c2h = np.asarray(w_c2h, np.float32)

    def gate_cols(wt):
        # wt: [in(64), out(192)] -> [in, 256] in [o|f|g|g] column order
        return np.concatenate([wt[:, 64:128], wt[:, 0:64],
                               wt[:, 128:192], wt[:, 128:192]], axis=1)

    w1t = conv1_w.transpose(1, 0, 2, 3)
    A1 = np.zeros((128, 6 * G2), np.float32)
    for k in range(3):
        A1[0:64, k * G2:(k + 1) * G2] = gate_cols(w1t[:, :, 0, k])
        A1[64:128, k * G2:(k + 1) * G2] = gate_cols(w1t[:, :, 1, k])
    for k in range(3, 6):
        A1[64:128, k * G2:(k + 1) * G2] = gate_cols(w1t[:, :, 2, k - 3])
    WL = np.zeros((128, 9 * G2), np.float32)
    wct = w_c2h.transpose(1, 0, 2, 3)
    wht = w_h2h.transpose(1, 0, 2, 3)
    for dy in range(3):
        for dx in range(3):
            tau = dy * 3 + dx
            WL[0:64, tau * G2:(tau + 1) * G2] = gate_cols(wht[:, :, dy, dx])
            WL[64:128, tau * G2:(tau + 1) * G2] = gate_cols(wct[:, :, dy, dx])
    b1 = np.concatenate([conv1_b[64:128], conv1_b[0:64],
                         conv1_b[128:192], conv1_b[128:192]])
    b1 = np.ascontiguousarray(b1.reshape(256, 1))
    gb = np.ascontiguousarray(
        np.stack([np.asarray(gamma, np.float32),
                  np.asarray(beta, np.float32)]))
    idm = np.zeros((128, 64), np.float32)
    idm[64:128] = np.eye(64, dtype=np.float32)

    in_maps = []
    for c in range(N_CORES):
        n, hh = c // 2, c % 2
        xs = np.zeros((T, C, PR, W), np.float32)
        if hh == 0:
            xs[:, :, 1:50, :] = x[:, n, :, 0:49, :]
        else:
            xs[:, :, 0:49, :] = x[:, n, :, 15:64, :]
        xst = np.ascontiguousarray(x[:, n, :, hh * 32:(hh + 1) * 32, :])
        msk2 = np.zeros((PR, PW), np.float32)
        if hh == 0:
            msk2[1:50, 1:65] = 1.0
        else:
            msk2[0:49, 1:65] = 1.0
        mflat = msk2.reshape(FT)
        msk = np.zeros((128, FT), np.float32)
        msk[0:64] = mflat
        msk[64:128, 0:FT - PW] = mflat[PW:]
        in_maps.append(dict(xs=xs, xst=xst, w1=A1, wl=WL, b1=b1, gb=gb,
                            msk=msk, idm=idm))
    return in_maps


_NC = None


def kernel(x, gamma, beta, conv1_w, conv1_b, w_h2h, w_c2h):
    global _NC
    in_maps = host_prep(x, gamma, beta, conv1_w, conv1_b, w_h2h, w_c2h)
    if _NC is None:
        _NC = build_nc()
    res = run_bass_kernel_spmd(_NC, in_maps, list(range(N_CORES)))
    out = np.zeros((T, NB, C, H, W), np.float32)
    for c in range(N_CORES):
        n, hh = c // 2, c % 2
        hs = res.results[c]["hs"]
        lo = 0 if hh == 0 else 16
        out[:, n, :, hh * 32:(hh + 1) * 32, :] = hs[:, :, lo:lo + 32, :]
    return out
